# revision 1
# baseline (speedup 1.0000x reference)
"""Causal self-attention (B=2, T=2048, C=1024, H=16) on 8 TRN2 NeuronCores.

Sharding: core c -> batch b = c//4, head-group g = c%4 (4 heads = 256 channels).
Each core computes its 4 heads end-to-end and a partial projection
(y_local @ W_proj[256g:256g+256, :]); the host sums the 4 partials per batch.

On-chip dataflow (matmuls fp32r = full-rate fp32, ~1.6e-4 rel err):
  qkT[ch, t]  = Wqkv[:, ch].T @ x[b].T          (q,k kept transposed: d on partitions)
  v[t, ch]    = x[b] @ Wv                       (natural layout, + ones column per head)
  S^T[k, q]   = k_h @ q_h^T  (per head, row-packed 2 heads per PE pass, K=64;
                diagonal chunks narrowed to their causally-valid column window)
  causal mask: short bf16 identity-matmul accumulates -1e30 onto the masked
                prefix of diagonal chunks (keeps mask work off DVE)
  P = exp(S^T) on ScalarE, one [128,1024] op per head pair
  y^T[d, q], denom[q] = [V_h | 1].T @ P        (ones column -> denominator row)
  y_norm^T = y^T * (1/denom)  (gpsimd partition_broadcast + DVE mult)
  out_partial[t, c] = y_norm^T.T @ W_proj_slice

Scheduling: engines execute streams in emission(priority) order, so qkv/proj
work is explicitly interleaved into the ACT-bound attention chunks (filler
queue), the input DMA ramp is filled with split-k first-half passes, and xT
streams in column halves so attention(0) unlocks after 7 of the 12 MB.
"""

import numpy as np

B, T, C = 2, 2048, 1024
H, HD = 16, 64
NCORES = 8
HEADS_PER_CORE = 4          # 2 pairs
CH = HEADS_PER_CORE * HD    # 256 channels per core
KT = C // 128               # 8 contraction tiles for qkv
NT = T // 128               # 16 key tiles / t tiles
NJ = T // 512               # 4 query chunks
SCALE = 1.0 / np.sqrt(HD)

_COMPILED = None  # (nc, names) cache


def _build():
    import concourse.bass as bass
    import concourse.bacc as bacc
    import concourse.mybir as mybir
    import concourse.tile as tile

    f32 = mybir.dt.float32
    f32r = mybir.dt.float32r
    r = lambda ap: ap.bitcast(f32r)

    nc = bacc.Bacc("TRN2", target_bir_lowering=False, debug=False)

    xT_d = nc.dram_tensor("xT", [C, T], f32, kind="ExternalInput").ap()
    wqkv_d = nc.dram_tensor("wqkv", [C, 3 * CH], f32, kind="ExternalInput").ap()
    bqk_d = nc.dram_tensor("bqk", [128, 4], f32, kind="ExternalInput").ap()
    bv_d = nc.dram_tensor("bv", [1, CH], f32, kind="ExternalInput").ap()
    wproj_d = nc.dram_tensor("wproj", [CH, C], f32, kind="ExternalInput").ap()
    bf16 = mybir.dt.bfloat16
    mask_d = nc.dram_tensor("mask", [128, 4 * 512], bf16, kind="ExternalInput").ap()
    ident_d = nc.dram_tensor("ident", [128, 128], bf16, kind="ExternalInput").ap()
    out_d = nc.dram_tensor("out_p", [T, C], f32, kind="ExternalOutput").ap()

    with tile.TileContext(nc) as tc:
        with (
            tc.tile_pool(name="p_w", bufs=1) as p_w,
            tc.tile_pool(name="p_x", bufs=1) as p_x,
            tc.tile_pool(name="p_qk", bufs=1) as p_qk,
            tc.tile_pool(name="p_v", bufs=1) as p_v,
            tc.tile_pool(name="p_y", bufs=1) as p_y,
            tc.tile_pool(name="p_p", bufs=3) as p_p,
            tc.tile_pool(name="p_sm", bufs=2) as p_sm,
            tc.tile_pool(name="ps_mm", bufs=2, space="PSUM") as ps_mm,
            tc.tile_pool(name="ps_s", bufs=4, space="PSUM") as ps_s,
            tc.tile_pool(name="ps_y", bufs=2, space="PSUM") as ps_y,
        ):
            # ---- persistent inputs -------------------------------------
            wqkv = [p_w.tile([128, 3 * CH], f32r, name=f"wqkv{k}", tag=f"wqkv{k}")
                    for k in range(KT)]
            xT = [p_x.tile([128, T], f32r, name=f"xT{k}", tag=f"xT{k}")
                  for k in range(KT)]
            wproj = [p_w.tile([128, C], f32r, name=f"wproj{k}", tag=f"wproj{k}")
                     for k in range(2)]
            mask = p_w.tile([128, 4 * 512], bf16, name="mask", tag="mask")
            ident = p_w.tile([128, 128], bf16, name="ident", tag="ident")
            bqk = p_w.tile([128, 4], f32, name="bqk", tag="bqk")
            bvrow = p_w.tile([1, CH], f32, name="bvrow", tag="bvrow")
            bvb = p_w.tile([128, CH], f32, name="bvb", tag="bvb")

            # t-columns 0:1024 of xT unlock waves 0/1 + attention(0);
            # the upper half is only needed by waves 2/3 and streams in later.
            for k in range(KT):
                nc.sync.dma_start(out=wqkv[k], in_=r(wqkv_d[128 * k:128 * (k + 1), :]))
                nc.sync.dma_start(out=xT[k][:, 0:1024],
                                  in_=r(xT_d[128 * k:128 * (k + 1), 0:1024]))
                if k == 1:
                    nc.sync.dma_start(out=mask, in_=mask_d)
                    nc.sync.dma_start(out=ident, in_=ident_d)
                    nc.sync.dma_start(out=bqk, in_=bqk_d)
                    nc.sync.dma_start(out=bvrow, in_=bv_d)
            for k in range(KT):
                nc.sync.dma_start(out=xT[k][:, 1024:2048],
                                  in_=r(xT_d[128 * k:128 * (k + 1), 1024:2048]))
            for k in range(2):
                nc.sync.dma_start(out=wproj[k], in_=r(wproj_d[128 * k:128 * (k + 1), :]))
            nc.gpsimd.partition_broadcast(bvb, bvrow[0:1, :])

            # ---- persistent intermediates ------------------------------
            # qT/kT: [128ch, T]; tile p holds heads (2p, 2p+1) on partitions 0:64/64:128
            qT = [p_qk.tile([128, T], f32r, name=f"qT{p}", tag=f"qT{p}") for p in range(2)]
            kT = [p_qk.tile([128, T], f32r, name=f"kT{p}", tag=f"kT{p}") for p in range(2)]
            # v tiles: [128 t, 4 heads * 65] (65th col of each head = 1.0)
            v = [p_v.tile([128, 4 * 65], f32r, name=f"v{m}", tag=f"v{m}") for m in range(NT)]
            # normalized y^T pair tiles
            yT = [p_y.tile([128, T], f32r, name=f"yT{p}", tag=f"yT{p}") for p in range(2)]

            def qkv_chunk(mi, nj, pool=None, tag=None):
                """qkv^T channels [128mi,128mi+128), t [512nj, 512nj+512)."""
                pool = pool or ps_mm
                ps = pool.tile([128, 512], f32, name="ps_qkv", tag=tag or "mm")
                for k in range(KT):
                    nc.tensor.matmul(
                        ps[:, 0:512],
                        lhsT=r(wqkv[k][:, 128 * mi:128 * (mi + 1)]),
                        rhs=r(xT[k][:, 512 * nj:512 * (nj + 1)]),
                        start=(k == 0), stop=(k == KT - 1),
                    )
                dst = qT[mi] if mi < 2 else kT[mi - 2]
                nc.vector.tensor_scalar_add(
                    dst[:, 512 * nj:512 * (nj + 1)], ps[:, 0:512], bqk[:, mi:mi + 1])

            def v_chunk(m):
                """v rows [128m, 128m+128), all 256 channels, into 65-strided tile."""
                ps = ps_mm.tile([128, 512], f32, name="ps_v", tag="mm")
                for k in range(KT):
                    nc.tensor.matmul(
                        ps[:, 0:CH],
                        lhsT=r(xT[k][:, 128 * m:128 * (m + 1)]),
                        rhs=r(wqkv[k][:, 2 * CH:3 * CH]),
                        start=(k == 0), stop=(k == KT - 1),
                    )
                for h in range(4):
                    nc.vector.memset(v[m][:, 65 * h + 64:65 * h + 65].bitcast(f32), 1.0)
                vi = v[m].rearrange("p (h c) -> p h c", h=4)[:, :, 0:64]
                nc.vector.tensor_tensor(
                    vi,
                    ps[:, 0:CH].rearrange("p (h c) -> p h c", h=4),
                    bvb.rearrange("p (h c) -> p h c", h=4),
                    mybir.AluOpType.add,
                )

            def qkv_chunk_split(mi, nj, half, pool, tag):
                ps = pool.tile([128, 512], f32, name="ps_qkvs", tag=tag)
                for k in range(4 * half, 4 * half + 4):
                    nc.tensor.matmul(
                        ps[:, 0:512],
                        lhsT=r(wqkv[k][:, 128 * mi:128 * (mi + 1)]),
                        rhs=r(xT[k][:, 512 * nj:512 * (nj + 1)]),
                        start=(k % 4 == 0), stop=(k % 4 == 3),
                    )
                dst = (qT[mi] if mi < 2 else kT[mi - 2])[:, 512 * nj:512 * (nj + 1)]
                if half == 0:
                    nc.vector.tensor_scalar_add(dst, ps[:, 0:512], bqk[:, mi:mi + 1])
                else:
                    nc.vector.tensor_tensor(dst, ps[:, 0:512], dst, mybir.AluOpType.add)

            def v_chunk_split(m, half, pool=None, tag=None):
                pool = pool or ps_mm
                ps = pool.tile([128, 512], f32, name="ps_vs", tag=tag or "mm")
                for k in range(4 * half, 4 * half + 4):
                    nc.tensor.matmul(
                        ps[:, 0:CH],
                        lhsT=r(xT[k][:, 128 * m:128 * (m + 1)]),
                        rhs=r(wqkv[k][:, 2 * CH:3 * CH]),
                        start=(k % 4 == 0), stop=(k % 4 == 3),
                    )
                vi = v[m].rearrange("p (h c) -> p h c", h=4)[:, :, 0:64]
                psv = ps[:, 0:CH].rearrange("p (h c) -> p h c", h=4)
                if half == 0:
                    for h in range(4):
                        nc.vector.memset(
                            v[m][:, 65 * h + 64:65 * h + 65].bitcast(f32), 1.0)
                    nc.vector.tensor_tensor(
                        vi, psv, bvb.rearrange("p (h c) -> p h c", h=4),
                        mybir.AluOpType.add)
                else:
                    nc.vector.tensor_tensor(vi, psv, vi, mybir.AluOpType.add)

            def attention(j, p, filler=None):
                """q-chunk j (512 queries), head pair p (heads 2p, 2p+1)."""
                ni = 4 * j + 4  # k-tiles 0..ni-1 are (partially) unmasked
                yA = ps_y.tile([128, 512], f32, name="yA", tag="y")
                yB = ps_y.tile([128, 512], f32, name="yB", tag="y")
                qs = slice(512 * j, 512 * (j + 1))
                for i in range(ni):
                    sA = ps_s.tile([128, 512], f32, name="s_a", tag="s")
                    sB = ps_s.tile([128, 512], f32, name="s_b", tag="s")
                    rr = i - 4 * j
                    diag = rr >= 0
                    # valid window for diagonal chunks: q >= 128*rr + k.
                    # W0 rounded down to 256 keeps the moving dim >= 256
                    # (full-rate fp32r); [W0:512) of each half is computed.
                    W0 = 0 if not diag else min(128 * rr, 256)
                    Wd = 512 - W0
                    qw = slice(512 * j + W0, 512 * (j + 1))
                    # S^T chunks for both heads, row-packed (K=64 each)
                    nc.tensor.matmul(
                        sA[:, W0:512],
                        lhsT=r(kT[p][0:64, 128 * i:128 * (i + 1)]),
                        rhs=r(qT[p][0:64, qw]),
                        start=True, stop=not diag,
                    )
                    nc.tensor.matmul(
                        sB[:, W0:512],
                        lhsT=r(kT[p][64:128, 128 * i:128 * (i + 1)]),
                        rhs=r(qT[p][64:128, qw]),
                        start=True, stop=not diag,
                    )
                    pt = p_p.tile([128, 1024], f32r, name="pt", tag="pt")
                    if diag:
                        # causal mask: short bf16 matmul accumulates -1e30 onto
                        # the masked prefix of the window
                        Wm = 128 * (rr + 1) - W0
                        for half, sh in ((0, sA), (1, sB)):
                            nc.tensor.matmul(
                                sh[:, W0:W0 + Wm],
                                lhsT=ident,
                                rhs=mask[:, 512 * rr:512 * rr + Wm],
                                start=False, stop=True,
                            )
                    for half, sh in ((0, sA), (1, sB)):
                        nc.scalar.activation(
                            pt[:, 512 * half + W0:512 * half + 512],
                            sh[:, W0:512],
                            mybir.ActivationFunctionType.Exp)
                    if filler is not None:
                        filler()
                    nc.tensor.matmul(
                        yA[0:65, W0:512],
                        lhsT=r(v[i][:, 65 * (2 * p):65 * (2 * p) + 65]),
                        rhs=r(pt[:, W0:512]),
                        start=(i == 0), stop=(i == ni - 1),
                    )
                    nc.tensor.matmul(
                        yB[0:65, W0:512],
                        lhsT=r(v[i][:, 65 * (2 * p + 1):65 * (2 * p + 1) + 65]),
                        rhs=r(pt[:, 512 + W0:1024]),
                        start=(i == 0), stop=(i == ni - 1),
                    )
                # normalize: row 64 of y psum = softmax denominator.
                # NB: partition_broadcast reads physical partition 0 on HW
                # (ignores AP partition offset) -> each recip gets its own tile.
                rcA = p_sm.tile([1, 512], f32, name="rcA", tag="rcA")
                rcB = p_sm.tile([1, 512], f32, name="rcB", tag="rcB")
                nc.vector.reciprocal(rcA, yA[64:65, :])
                nc.vector.reciprocal(rcB, yB[64:65, :])
                bcA = p_sm.tile([64, 512], f32, name="bcA", tag="bcA")
                bcB = p_sm.tile([64, 512], f32, name="bcB", tag="bcB")
                nc.gpsimd.partition_broadcast(bcA, rcA[0:1, :])
                nc.gpsimd.partition_broadcast(bcB, rcB[0:1, :])
                nc.vector.tensor_tensor(
                    yT[p][0:64, qs], yA[0:64, :], bcA, mybir.AluOpType.mult)
                nc.vector.tensor_tensor(
                    yT[p][64:128, qs], yB[0:64, :], bcB, mybir.AluOpType.mult)

            def proj(m):
                """output rows [128m, 128m+128)."""
                for u in range(2):
                    if u == 0:
                        ps = ps_s.tile([128, 512], f32, name="ps_pr", tag="s")
                    else:
                        ps = ps_mm.tile([128, 512], f32, name="ps_pr2", tag="mm")
                    for kk in range(2):
                        nc.tensor.matmul(
                            ps[:, 0:512],
                            lhsT=r(yT[kk][:, 128 * m:128 * (m + 1)]),
                            rhs=r(wproj[kk][:, 512 * u:512 * (u + 1)]),
                            start=(kk == 0), stop=(kk == 1),
                        )
                    st = p_p.tile([128, 512], f32, name="st_pr", tag="st_pr", bufs=6)
                    if u == 0:
                        nc.vector.tensor_copy(st, ps[:, 0:512])
                        eng = nc.sync
                    else:
                        nc.scalar.copy(st, ps[:, 0:512])
                        eng = nc.gpsimd
                    eng.dma_start(
                        out=out_d[128 * m:128 * (m + 1), 512 * u:512 * (u + 1)],
                        in_=st)

            # ---- emission order (scheduling priority) -------------------
            # Engines execute their instruction streams in emission (priority)
            # order, so prefetch work must be explicitly interleaved into the
            # ACT-bound attention chunks via a filler queue.
            # ramp: first halves of waves 0+1 run while x4..7 stream in;
            # wave-0 second halves unlock attention(0); wave-1 second halves
            # become the j=0 fillers.
            for nj in (0, 1):
                for mi in (0, 2):
                    qkv_chunk_split(mi, nj, 0, ps_s, "s")
                for m in range(4 * nj, 4 * nj + 4):
                    v_chunk_split(m, 0)
                for mi in (1, 3):
                    qkv_chunk_split(mi, nj, 0, ps_s, "s")
            for mi in (0, 2):
                qkv_chunk_split(mi, 0, 1, ps_s, "s")
            for m in range(4):
                v_chunk_split(m, 1)
            for mi in (1, 3):
                qkv_chunk_split(mi, 0, 1, ps_s, "s")
            for mi in (0, 2, 1, 3):
                qkv_chunk_split(mi, 2, 0, ps_mm, "mm")
            for m in range(8, 12):
                v_chunk_split(m, 0)

            fillers = []

            def filler():
                if fillers:
                    fillers.pop(0)()

            for j in range(NJ):
                if j == 0:  # wave-1 second halves (firsts ran in the ramp)
                    for mi in (0, 2):
                        fillers.append(
                            lambda mi=mi: qkv_chunk_split(mi, 1, 1, ps_mm, "mm"))
                    for m in range(4, 8):
                        fillers.append(lambda m=m: v_chunk_split(m, 1))
                    for mi in (1, 3):
                        fillers.append(
                            lambda mi=mi: qkv_chunk_split(mi, 1, 1, ps_mm, "mm"))
                elif j == 1:  # wave-2 second halves (firsts ran in the ramp)
                    for mi in (0, 2):
                        fillers.append(
                            lambda mi=mi: qkv_chunk_split(mi, 2, 1, ps_mm, "mm"))
                    for m in range(8, 12):
                        fillers.append(lambda m=m: v_chunk_split(m, 1))
                    for mi in (1, 3):
                        fillers.append(
                            lambda mi=mi: qkv_chunk_split(mi, 2, 1, ps_mm, "mm"))
                elif j + 1 < NJ:  # next wave's qkv/v chunks, as fillers
                    for mi in (0, 2, 1, 3):
                        fillers.append(lambda mi=mi, nj=j + 1: qkv_chunk(mi, nj))
                    for m in range(4 * (j + 1), 4 * (j + 2)):
                        fillers.append(lambda m=m: v_chunk(m))
                if j > 0:  # previous chunk's projection: half now, half next j
                    lo = 4 * (j - 1)
                    for m in range(lo, lo + (2 if j < 3 else 4)):
                        fillers.append(lambda m=m: proj(m))
                if j == 3:  # deferred halves of proj(0), proj(1)
                    for m in (2, 3, 6, 7):
                        fillers.append(lambda m=m: proj(m))
                for p in range(2):
                    attention(j, p, filler)
                # drain what the chunks could not absorb before the boundary
                while fillers:
                    fillers.pop(0)()
            for m in range(12, 16):
                proj(m)

    nc.compile()
    return nc


def _host_inputs(x, W_attn, b_attn, W_proj):
    """Build the 8 per-core input maps (numpy only)."""
    x = np.asarray(x, dtype=np.float32)
    W_attn = np.asarray(W_attn, dtype=np.float32)
    b_attn = np.asarray(b_attn, dtype=np.float32)
    W_proj = np.asarray(W_proj, dtype=np.float32)

    import ml_dtypes
    # additive causal masks, windowed: for diag offset r the S chunk is
    # computed on columns [W0, 512) (W0 = min(128r, 256)); the mask pattern at
    # offset 512r covers the masked prefix q' < 128r + k - W0 of that window.
    kl = np.arange(128)[:, None]
    blocks = []
    for rr in range(4):
        W0 = min(128 * rr, 256)
        qp = np.arange(512)[None, :] + W0
        blocks.append(np.where(qp >= kl + 128 * rr, 0.0, -1e30))
    mask = np.concatenate(blocks, axis=1).astype(ml_dtypes.bfloat16)
    ident = np.eye(128, dtype=ml_dtypes.bfloat16)

    in_maps = []
    for c in range(NCORES):
        b, g = divmod(c, 4)
        sl = slice(CH * g, CH * (g + 1))
        wq = W_attn[:, 0 * C:1 * C][:, sl] * SCALE
        wk = W_attn[:, 1 * C:2 * C][:, sl]
        wv = W_attn[:, 2 * C:3 * C][:, sl]
        bq = b_attn[0 * C:1 * C][sl] * SCALE
        bk = b_attn[1 * C:2 * C][sl]
        bv = b_attn[2 * C:3 * C][sl]
        bqk = np.stack([bq[0:128], bq[128:256], bk[0:128], bk[128:256]], axis=1)
        in_maps.append({
            "xT": np.ascontiguousarray(x[b].T),
            "wqkv": np.ascontiguousarray(np.concatenate([wq, wk, wv], axis=1)),
            "bqk": np.ascontiguousarray(bqk),
            "bv": np.ascontiguousarray(bv[None, :]),
            "wproj": np.ascontiguousarray(W_proj[sl, :]),
            "mask": mask,
            "ident": ident,
        })
    return in_maps


def kernel(x, W_attn, b_attn, W_proj, b_proj, _want_results=None):
    global _COMPILED
    from concourse.bass_utils import run_bass_kernel_spmd

    if _COMPILED is None:
        _COMPILED = _build()
    nc = _COMPILED

    in_maps = _host_inputs(x, W_attn, b_attn, W_proj)
    kw = dict(_want_results or {})
    res = run_bass_kernel_spmd(nc, in_maps, core_ids=list(range(NCORES)), **kw)
    if _want_results is not None:
        kernel.last_results = res

    out = np.zeros((B, T, C), dtype=np.float32)
    for c in range(NCORES):
        out[c // 4] += res.results[c]["out_p"]
    out += np.asarray(b_proj, dtype=np.float32)[None, None, :]
    return out



# revision 8
# speedup vs baseline: 1.0799x; 1.0799x over previous
"""Causal self-attention (B=2, T=2048, C=1024, H=16) on 8 TRN2 NeuronCores.

Sharding: core c -> batch b = c//4, head-group g = c%4 (4 heads = 256 channels).
Each core computes its 4 heads end-to-end and a scaled partial projection
(1024 * y_norm_local @ W_proj[256g:+256, :] in fp16); the host rescales and
sums the 4 partials per batch.

Mixed-precision dataflow (PE fp8 DoubleRow = 0.5 cyc/col, fp16 = 1 cyc/col):
  host:  xh = e4m3(x^T), xl = e4m3((x^T - xh)*16)     (split-fp8 activations)
         Wqk = e4m3(32*W), Wvh/Wvl/Wvh16 split-fp8, Wp = fp16(32*Wp)
  qk:    q32[128ch, t] = Wqk_pair.T (x) xh_pair       (1-pass fp8 DoubleRow)
         -> bias-add-cast to fp8 pair tiles [128, 2, T] (i=1 zeroed; two
            heads packed on partition halves, d on partitions)
  v:     v32[t, ch] = xh.Wvh + xh.Wvl + xl.Wvh16      (3-pass fp8 DoubleRow)
         -> fp16 v tiles [128t, 4h*65] (65th col = 1.0 -> softmax denom)
  S^T:   per head pair-of-keytiles psum [128k, 2, 512q]: fp8 DoubleRow
         (K=64 on partitions 0:64 / 64:128 + zero half), exact causal
         windows W0 = 128*rr; bf16 identity-matmul masks (-1e30)
  P:     one exp per pair: [128, (2, W)] psum -> fp16 P tile (scale 2^-13)
  PV:    y[65, q] psum accumulated per keytile, fp16 (ones row = denom)
  norm:  recip (DVE) + partition_broadcast (Pool) + mult -> yT fp16 [128, T]
  proj:  out[t, c] = yT.T @ Wp16, fp16 staging -> fp16 partials to host

Scheduling: engines execute streams in emission(priority) order.  The
S->exp->PV chain is software-pipelined one pair deep (emit S(k+1) before
PV(k)) so PE never head-of-line blocks on ACT exp; qkv/v waves for j+1 and
proj(j-1) interleave as fillers between pairs.
"""

import numpy as np

B, T, C = 2, 2048, 1024
H, HD = 16, 64
NCORES = 8
HEADS_PER_CORE = 4
CH = HEADS_PER_CORE * HD    # 256 channels per core
NKK = 4                     # 256-channel contraction pair-tiles for qkv
NT = T // 128               # 16 key tiles
NJ = T // 512               # 4 query chunks

_COMPILED = None


def _build():
    import concourse.bass as bass
    import concourse.bacc as bacc
    import concourse.mybir as mybir
    import concourse.tile as tile

    f32 = mybir.dt.float32
    f16 = mybir.dt.float16
    f8 = mybir.dt.float8e4
    bf16 = mybir.dt.bfloat16
    DR = mybir.MatmulPerfMode.DoubleRow
    EXP_SCALE = float(2.0 ** -13)   # 1/(sqrt(64) * 32 * 32)

    nc = bacc.Bacc("TRN2", target_bir_lowering=False, debug=False)

    xh_d = nc.dram_tensor("xh", [C, T], f8, kind="ExternalInput").ap()
    xl_d = nc.dram_tensor("xl", [C, T], f8, kind="ExternalInput").ap()
    wqk_d = nc.dram_tensor("wqk", [C, 2 * CH], f8, kind="ExternalInput").ap()
    wvh_d = nc.dram_tensor("wvh", [C, CH], f8, kind="ExternalInput").ap()
    wvl_d = nc.dram_tensor("wvl", [C, CH], f8, kind="ExternalInput").ap()
    wvh16_d = nc.dram_tensor("wvh16", [C, CH], f8, kind="ExternalInput").ap()
    wp_d = nc.dram_tensor("wp", [CH, C], f16, kind="ExternalInput").ap()
    bqk_d = nc.dram_tensor("bqk", [128, 4], f32, kind="ExternalInput").ap()
    bv_d = nc.dram_tensor("bv", [1, CH], f32, kind="ExternalInput").ap()
    tri_d = nc.dram_tensor("tri", [128, 128], bf16, kind="ExternalInput").ap()
    neg_d = nc.dram_tensor("neg", [128, 128], bf16, kind="ExternalInput").ap()
    id_d = nc.dram_tensor("id", [128, 128], bf16, kind="ExternalInput").ap()
    zero_d = nc.dram_tensor("zero", [128, T], f8, kind="ExternalInput").ap()
    out_d = nc.dram_tensor("out_p", [T, C], f16, kind="ExternalOutput").ap()

    with tile.TileContext(nc) as tc:
        with (
            tc.tile_pool(name="p_w", bufs=1) as p_w,
            tc.tile_pool(name="p_x", bufs=1) as p_x,
            tc.tile_pool(name="p_qk", bufs=1) as p_qk,
            tc.tile_pool(name="p_v", bufs=1) as p_v,
            tc.tile_pool(name="p_y", bufs=1) as p_y,
            tc.tile_pool(name="p_p", bufs=3) as p_p,
            tc.tile_pool(name="p_sm", bufs=2) as p_sm,
            tc.tile_pool(name="p_st", bufs=6) as p_st,
            tc.tile_pool(name="ps_s", bufs=2, space="PSUM") as ps_s,
            tc.tile_pool(name="ps_y", bufs=2, space="PSUM") as ps_y,
            tc.tile_pool(name="ps_mm", bufs=2, space="PSUM") as ps_mm,
        ):
            # ---- persistent inputs -------------------------------------
            xh = [p_x.tile([128, 2, T], f8, name=f"xh{k}", tag=f"xh{k}")
                  for k in range(NKK)]
            xl = [p_x.tile([128, 2, T], f8, name=f"xl{k}", tag=f"xl{k}")
                  for k in range(NKK)]
            wqk = [p_w.tile([128, 2, 2 * CH], f8, name=f"wqk{k}", tag=f"wqk{k}")
                   for k in range(NKK)]
            wvh = [p_w.tile([128, 2, CH], f8, name=f"wvh{k}", tag=f"wvh{k}")
                   for k in range(NKK)]
            wvl = [p_w.tile([128, 2, CH], f8, name=f"wvl{k}", tag=f"wvl{k}")
                   for k in range(NKK)]
            wvh16 = [p_w.tile([128, 2, CH], f8, name=f"wvh16{k}", tag=f"wvh16{k}")
                     for k in range(NKK)]
            wp = [p_w.tile([128, C], f16, name=f"wp{k}", tag=f"wp{k}")
                  for k in range(2)]
            tri = p_w.tile([128, 128], bf16, name="tri", tag="tri")
            neg = p_w.tile([128, 128], bf16, name="neg", tag="neg")
            ident = p_w.tile([128, 128], bf16, name="ident", tag="ident")
            bqk = p_w.tile([128, 4], f32, name="bqk", tag="bqk")
            bvrow = p_w.tile([1, CH], f32, name="bvrow", tag="bvrow")
            bvb = p_w.tile([128, CH], f32, name="bvb", tag="bvb")

            # ---- persistent intermediates ------------------------------
            # q/k fp8 pair tiles: [128, 2, T]; heads (2p, 2p+1) packed on
            # partitions 0:64 / 64:128; i=1 half zeroed (DoubleRow padding).
            qt = [p_qk.tile([128, 2, T], f8, name=f"qt{p}", tag=f"qt{p}")
                  for p in range(2)]
            kt = [p_qk.tile([128, 2, T], f8, name=f"kt{p}", tag=f"kt{p}")
                  for p in range(2)]
            # v tiles: [128t, 4 heads * 65] fp16 (65th col of each head = 1)
            v = [p_v.tile([128, 4 * 65], f16, name=f"v{m}", tag=f"v{m}")
                 for m in range(NT)]
            # normalized y^T fp16: tile kk holds heads (2kk, 2kk+1)
            yT = [p_y.tile([128, T], f16, name=f"yT{p}", tag=f"yT{p}")
                  for p in range(2)]

            # ---- input DMA ramp ---------------------------------------
            # t-columns 0:1024 of xh unlock wave 0/1 + attention(0).
            for k in range(NKK):
                for i in range(2):
                    nc.sync.dma_start(
                        out=xh[k][:, i, 0:1024],
                        in_=xh_d[256 * k + 128 * i:256 * k + 128 * (i + 1), 0:1024])
            for k in range(NKK):
                for i in range(2):
                    nc.sync.dma_start(
                        out=wqk[k][:, i, :],
                        in_=wqk_d[256 * k + 128 * i:256 * k + 128 * (i + 1), :])
                if k == 0:
                    nc.sync.dma_start(out=tri, in_=tri_d)
                    nc.sync.dma_start(out=neg, in_=neg_d)
                    nc.sync.dma_start(out=ident, in_=id_d)
                    nc.sync.dma_start(out=bqk, in_=bqk_d)
                    nc.sync.dma_start(out=bvrow, in_=bv_d)
            # zero the i=1 halves of q/k pair tiles (DoubleRow zero padding)
            for t_ in (qt[0], qt[1], kt[0], kt[1]):
                nc.sync.dma_start(out=t_[:, 1, :], in_=zero_d)
            for k in range(NKK):
                for i in range(2):
                    nc.sync.dma_start(
                        out=wvh[k][:, i, :],
                        in_=wvh_d[256 * k + 128 * i:256 * k + 128 * (i + 1), :])
                    nc.sync.dma_start(
                        out=wvl[k][:, i, :],
                        in_=wvl_d[256 * k + 128 * i:256 * k + 128 * (i + 1), :])
                    nc.sync.dma_start(
                        out=wvh16[k][:, i, :],
                        in_=wvh16_d[256 * k + 128 * i:256 * k + 128 * (i + 1), :])
            for k in range(NKK):
                for i in range(2):
                    nc.sync.dma_start(
                        out=xl[k][:, i, 0:1024],
                        in_=xl_d[256 * k + 128 * i:256 * k + 128 * (i + 1), 0:1024])
            for k in range(NKK):
                for i in range(2):
                    nc.sync.dma_start(
                        out=xh[k][:, i, 1024:2048],
                        in_=xh_d[256 * k + 128 * i:256 * k + 128 * (i + 1), 1024:2048])
            for k in range(NKK):
                for i in range(2):
                    nc.sync.dma_start(
                        out=xl[k][:, i, 1024:2048],
                        in_=xl_d[256 * k + 128 * i:256 * k + 128 * (i + 1), 1024:2048])
            for k in range(2):
                nc.sync.dma_start(out=wp[k], in_=wp_d[128 * k:128 * (k + 1), :])
            nc.gpsimd.partition_broadcast(bvb, bvrow[0:1, :])

            # ---- building blocks --------------------------------------
            def qk_chunk(mi, nj):
                """q or k channels [128mi, 128mi+128), t [512nj, +512).
                mi 0/1 -> q pair tiles, 2/3 -> k pair tiles."""
                ps = ps_mm.tile([128, 512], f32, name="ps_qk", tag="mm")
                for k in range(NKK):
                    nc.tensor.matmul(
                        ps[:, 0:512],
                        lhsT=wqk[k][:, :, 128 * mi:128 * (mi + 1)],
                        rhs=xh[k][:, :, 512 * nj:512 * (nj + 1)],
                        start=(k == 0), stop=(k == NKK - 1), perf_mode=DR)
                dst = (qt[mi] if mi < 2 else kt[mi - 2])
                nc.vector.tensor_scalar_add(
                    dst[:, 0, 512 * nj:512 * (nj + 1)], ps[:, 0:512],
                    bqk[:, mi:mi + 1])

            def v_chunk(m):
                """v rows [128m, +128), all 256 channels, 3-pass split fp8."""
                ps = ps_mm.tile([128, 512], f32, name="ps_v", tag="mm")
                for k in range(NKK):
                    nc.tensor.matmul(
                        ps[:, 0:CH],
                        lhsT=xh[k][:, :, 128 * m:128 * (m + 1)],
                        rhs=wvh[k], start=(k == 0), stop=False, perf_mode=DR)
                for k in range(NKK):
                    nc.tensor.matmul(
                        ps[:, 0:CH],
                        lhsT=xh[k][:, :, 128 * m:128 * (m + 1)],
                        rhs=wvl[k], start=False, stop=False, perf_mode=DR)
                for k in range(NKK):
                    nc.tensor.matmul(
                        ps[:, 0:CH],
                        lhsT=xl[k][:, :, 128 * m:128 * (m + 1)],
                        rhs=wvh16[k], start=False, stop=(k == NKK - 1),
                        perf_mode=DR)
                for h in range(4):
                    nc.vector.memset(v[m][:, 65 * h + 64:65 * h + 65], 1.0)
                vi = v[m].rearrange("p (h c) -> p h c", h=4)[:, :, 0:64]
                nc.vector.tensor_tensor(
                    vi, ps[:, 0:CH].rearrange("p (h c) -> p h c", h=4),
                    bvb.rearrange("p (h c) -> p h c", h=4), mybir.AluOpType.add)

            yps = {}     # (j, h) -> y psum tile

            def s_exp(j, h, mp):
                """S^T + exp for head h, q-chunk j, keytile pair (2mp, 2mp+1).
                Returns (P tile, W0 of the pair window)."""
                p2, hi = divmod(h, 2)       # pair tile index, half
                qs0, qs1 = 64 * hi, 64 * (hi + 1)
                sps = ps_s.tile([128, 2, 512], f32, name="sps", tag="s")
                rr0 = 2 * mp - 4 * j        # diag offset of first tile (<0 if off-diag)
                W = [0, 0]                  # exact causal window starts
                for u in range(2):
                    i = 2 * mp + u
                    rr = i - 4 * j
                    W[u] = max(0, 128 * rr)
                    junk = rr >= 0 and W[u] > W[0]
                    if junk:
                        # the pair-exp window starts at W[0]; initialize the
                        # causally-dead prefix [W[0], W[u]) of this bank with
                        # -1e30 (opens the bank's psum group)
                        nc.tensor.matmul(
                            sps[:, u, W[0]:W[u]],
                            lhsT=ident[:, 0:128], rhs=neg[:, 0:W[u] - W[0]],
                            start=True, stop=False)
                    nc.tensor.matmul(
                        sps[:, u, W[u]:512],
                        lhsT=kt[p2][qs0:qs1, :, 128 * i:128 * (i + 1)],
                        rhs=qt[p2][qs0:qs1, :, 512 * j + W[u]:512 * (j + 1)],
                        start=not junk, stop=(rr < 0), perf_mode=DR)
                    if rr >= 0:
                        # triangular mask on the causal boundary block
                        nc.tensor.matmul(
                            sps[:, u, W[u]:W[u] + 128],
                            lhsT=ident, rhs=tri,
                            start=False, stop=True)
                W0 = W[0]
                pt = p_p.tile([128, 2, 512], f16, name="pt", tag="pt")
                nc.scalar.activation(
                    pt[:, :, W0:512], sps[:, :, W0:512],
                    mybir.ActivationFunctionType.Exp, scale=EXP_SCALE)
                return pt, W

            def pv(j, h, mp, pt, W, last):
                """accumulate y psum for (j, h) from P pair; normalize if last."""
                if mp == 0:
                    yps[(j, h)] = ps_y.tile([65, 512], f32, name="yp", tag="y")
                yp = yps[(j, h)]
                for u in range(2):
                    i = 2 * mp + u
                    nc.tensor.matmul(
                        yp[:, W[u]:512],
                        lhsT=v[i][:, 65 * h:65 * h + 65],
                        rhs=pt[:, u, W[u]:512],
                        start=(i == 0), stop=(last and u == 1))
                if last:
                    rc = p_sm.tile([1, 512], f32, name="rc", tag=f"rc{h % 2}")
                    bc = p_sm.tile([64, 512], f32, name="bc", tag=f"bc{h % 2}")
                    nc.vector.reciprocal(rc, yp[64:65, :])
                    nc.gpsimd.partition_broadcast(bc, rc[0:1, :])
                    p2, hi = divmod(h, 2)
                    nc.vector.tensor_tensor(
                        yT[p2][64 * hi:64 * (hi + 1), 512 * j:512 * (j + 1)],
                        yp[0:64, :], bc, mybir.AluOpType.mult)

            def proj(m):
                """output rows [128m, +128): 2 matmuls per 512-col half."""
                st = p_st.tile([128, 1024], f16, name="st", tag="st")
                for u in range(2):
                    ps = ps_mm.tile([128, 512], f32, name="ps_pr", tag="mm")
                    for kk in range(2):
                        nc.tensor.matmul(
                            ps[:, 0:512],
                            lhsT=yT[kk][:, 128 * m:128 * (m + 1)],
                            rhs=wp[kk][:, 512 * u:512 * (u + 1)],
                            start=(kk == 0), stop=(kk == 1))
                    if u == 0:
                        nc.vector.tensor_copy(st[:, 0:512], ps[:, 0:512])
                    else:
                        nc.scalar.copy(st[:, 512:1024], ps[:, 0:512])
                eng = nc.sync if m % 2 == 0 else nc.gpsimd
                eng.dma_start(out=out_d[128 * m:128 * (m + 1), :], in_=st)

            # ---- emission schedule ------------------------------------
            fillers = []

            def fill():
                if fillers:
                    fillers.pop(0)()

            # wave 0 (q/k chunks then v tiles 0..3) ahead of attention(0)
            for mi in (0, 2, 1, 3):
                qk_chunk(mi, 0)
            for m in range(4):
                v_chunk(m)

            PAIRS = [(j, h, mp)
                     for j in range(NJ) for h in range(4) for mp in range(2 * j + 2)]
            pending = None
            cur_j = -1
            for (j, h, mp) in PAIRS:
                if j != cur_j:
                    while fillers:
                        fillers.pop(0)()
                    cur_j = j
                    if j + 1 < NJ:   # next wave's qkv as fillers
                        for mi in (0, 2, 1, 3):
                            fillers.append(
                                lambda mi=mi, nj=j + 1: qk_chunk(mi, nj))
                        for m in range(4 * (j + 1), 4 * (j + 2)):
                            fillers.append(lambda m=m: v_chunk(m))
                    if j >= 1:       # proj of the previous chunk
                        for m in range(4 * (j - 1), 4 * j):
                            fillers.append(lambda m=m: proj(m))
                pt_w = s_exp(j, h, mp)
                if pending is not None:
                    pj, ph, pmp, ppt, pW = pending
                    pv(pj, ph, pmp, ppt, pW, last=(pmp == 2 * pj + 1))
                fill()
                pending = (j, h, mp, pt_w[0], pt_w[1])
            pj, ph, pmp, ppt, pW = pending
            pv(pj, ph, pmp, ppt, pW, last=True)
            while fillers:
                fillers.pop(0)()
            for m in range(12, 16):
                proj(m)

    nc.compile()
    return nc


def _host_inputs(x, W_attn, b_attn, W_proj):
    """Build the 8 per-core input maps (numpy only)."""
    import ml_dtypes
    e4 = ml_dtypes.float8_e4m3
    bf = ml_dtypes.bfloat16

    x = np.asarray(x, dtype=np.float32)
    W_attn = np.asarray(W_attn, dtype=np.float32)
    b_attn = np.asarray(b_attn, dtype=np.float32)
    W_proj = np.asarray(W_proj, dtype=np.float32)

    # causal triangular block: [k_local p, q_local c] = 0 if c >= p else -1e30
    cc = np.arange(128)[None, :]
    pp = np.arange(128)[:, None]
    tri = np.where(cc >= pp, 0.0, -1e30).astype(bf)
    neg = np.full((128, 128), -1e30, np.float32).astype(bf)
    ident = np.eye(128, dtype=bf)
    zero = np.zeros((128, T), dtype=e4)

    in_maps = []
    for c in range(NCORES):
        b, g = divmod(c, 4)
        sl = slice(CH * g, CH * (g + 1))
        xb = np.ascontiguousarray(x[b].T)                    # [C, T]
        xh = xb.astype(e4)
        xl = ((xb - xh.astype(np.float32)) * 16.0).astype(e4)
        wq = 32.0 * W_attn[:, 0 * C:1 * C][:, sl]
        wk = 32.0 * W_attn[:, 1 * C:2 * C][:, sl]
        wv = 32.0 * W_attn[:, 2 * C:3 * C][:, sl]
        wqk = np.concatenate([wq, wk], axis=1).astype(e4)
        wvh = wv.astype(e4)
        wvl = (wv - wvh.astype(np.float32)).astype(e4)
        wvh16 = (wvh.astype(np.float32) / 16.0).astype(e4)
        wp = (32.0 * W_proj[sl, :]).astype(np.float16)
        bq = 32.0 * b_attn[0 * C:1 * C][sl]
        bk = 32.0 * b_attn[1 * C:2 * C][sl]
        bv = 32.0 * b_attn[2 * C:3 * C][sl]
        bqk = np.stack([bq[0:128], bq[128:256], bk[0:128], bk[128:256]], axis=1)
        in_maps.append({
            "xh": xh, "xl": xl,
            "wqk": np.ascontiguousarray(wqk),
            "wvh": np.ascontiguousarray(wvh),
            "wvl": np.ascontiguousarray(wvl),
            "wvh16": np.ascontiguousarray(wvh16),
            "wp": np.ascontiguousarray(wp),
            "bqk": np.ascontiguousarray(bqk.astype(np.float32)),
            "bv": np.ascontiguousarray(bv[None, :].astype(np.float32)),
            "tri": tri, "neg": neg, "id": ident, "zero": zero,
        })
    return in_maps


def kernel(x, W_attn, b_attn, W_proj, b_proj, _want_results=None):
    global _COMPILED
    from concourse.bass_utils import run_bass_kernel_spmd

    if _COMPILED is None:
        _COMPILED = _build()
    nc = _COMPILED

    in_maps = _host_inputs(x, W_attn, b_attn, W_proj)
    kw = dict(_want_results or {})
    res = run_bass_kernel_spmd(nc, in_maps, core_ids=list(range(NCORES)), **kw)
    if _want_results is not None:
        kernel.last_results = res

    out = np.zeros((B, T, C), dtype=np.float32)
    for c in range(NCORES):
        out[c // 4] += np.asarray(res.results[c]["out_p"], dtype=np.float32)
    out *= 1.0 / 1024.0
    out += np.asarray(b_proj, dtype=np.float32)[None, None, :]
    return out


# revision 9
# speedup vs baseline: 1.2702x; 1.1762x over previous
"""Causal self-attention (B=2, T=2048, C=1024, H=16) on 8 TRN2 NeuronCores.

Sharding: core c -> batch b = c//4, head-group g = c%4 (4 heads = 256 channels).
Each core computes its 4 heads end-to-end and a scaled partial projection
(1024 * y_norm_local @ W_proj[256g:+256, :] in fp16); the host rescales and
sums the 4 partials per batch.

Mixed-precision dataflow (PE fp8 DoubleRow = 0.5 cyc/col, fp16 = 1 cyc/col):
  host:  xh = e4m3(x^T), xl = e4m3((x^T - xh)*16)     (split-fp8 activations)
         Wqk = e4m3(32*W), Wvh/Wvl/Wvh16 split-fp8, Wp = fp16(32*Wp)
  qk:    q32[128ch, t] = Wqk_pair.T (x) xh_pair       (1-pass fp8 DoubleRow)
         -> bias-add-cast to fp8 pair tiles [128, 2, T] (i=1 zeroed; two
            heads packed on partition halves, d on partitions)
  v:     v32[t, ch] = xh.Wvh + xh.Wvl + xl.Wvh16      (3-pass fp8 DoubleRow)
         -> fp16 v tiles [128t, 4h*65] (65th col = 1.0 -> softmax denom)
  S^T:   per head pair-of-keytiles psum [128k, 2, 512q]: fp8 DoubleRow
         (K=64 on partitions 0:64 / 64:128 + zero half), exact causal
         windows W0 = 128*rr; bf16 identity-matmul masks (-1e30)
  P:     one exp per pair: [128, (2, W)] psum -> fp16 P tile (scale 2^-13)
  PV:    y[65, q] psum accumulated per keytile, fp16 (ones row = denom)
  norm:  recip (DVE) + partition_broadcast (Pool) + mult -> yT fp16 [128, T]
  proj:  out[t, c] = yT.T @ Wp16, fp16 staging -> fp16 partials to host

Scheduling: engines execute streams in emission(priority) order.  The
S->exp->PV chain is software-pipelined one pair deep (emit S(k+1) before
PV(k)) so PE never head-of-line blocks on ACT exp; qkv/v waves for j+1 and
proj(j-1) interleave as fillers between pairs.
"""

import numpy as np

B, T, C = 2, 2048, 1024
H, HD = 16, 64
NCORES = 8
HEADS_PER_CORE = 4
CH = HEADS_PER_CORE * HD    # 256 channels per core
NKK = 4                     # 256-channel contraction pair-tiles for qkv
NT = T // 128               # 16 key tiles
NJ = T // 512               # 4 query chunks

_COMPILED = None


def _build():
    import concourse.bass as bass
    import concourse.bacc as bacc
    import concourse.mybir as mybir
    import concourse.tile as tile

    f32 = mybir.dt.float32
    f16 = mybir.dt.float16
    f8 = mybir.dt.float8e4
    bf16 = mybir.dt.bfloat16
    DR = mybir.MatmulPerfMode.DoubleRow
    EXP_SCALE = float(2.0 ** -13)   # 1/(sqrt(64) * 32 * 32)

    nc = bacc.Bacc("TRN2", target_bir_lowering=False, debug=False)

    xh_d = nc.dram_tensor("xh", [C, T], f8, kind="ExternalInput").ap()
    xl_d = nc.dram_tensor("xl", [C, T], f8, kind="ExternalInput").ap()
    wqk_d = nc.dram_tensor("wqk", [C, 2 * CH], f8, kind="ExternalInput").ap()
    wv3_d = nc.dram_tensor("wv3", [C, 3 * CH], f8, kind="ExternalInput").ap()
    wp_d = nc.dram_tensor("wp", [CH, C], f16, kind="ExternalInput").ap()
    msk_d = nc.dram_tensor("msk", [128, 3 * 128], bf16, kind="ExternalInput").ap()
    misc_d = nc.dram_tensor("misc", [128, 264], f32, kind="ExternalInput").ap()
    zero_d = nc.dram_tensor("zero", [128, 2 * T], f8, kind="ExternalInput").ap()
    out_d = nc.dram_tensor("out_p", [T, C], f16, kind="ExternalOutput").ap()

    with tile.TileContext(nc) as tc:
        with (
            tc.tile_pool(name="p_w", bufs=1) as p_w,
            tc.tile_pool(name="p_x", bufs=1) as p_x,
            tc.tile_pool(name="p_qk", bufs=1) as p_qk,
            tc.tile_pool(name="p_v", bufs=1) as p_v,
            tc.tile_pool(name="p_y", bufs=1) as p_y,
            tc.tile_pool(name="p_p", bufs=3) as p_p,
            tc.tile_pool(name="p_sm", bufs=2) as p_sm,
            tc.tile_pool(name="p_st", bufs=6) as p_st,
            tc.tile_pool(name="ps_s", bufs=2, space="PSUM") as ps_s,
            tc.tile_pool(name="ps_y", bufs=2, space="PSUM") as ps_y,
            tc.tile_pool(name="ps_mm", bufs=2, space="PSUM") as ps_mm,
        ):
            # ---- persistent inputs (consolidated: one DMA per tensor) --
            xh_t = p_x.tile([128, NKK, 2, T], f8, name="xh", tag="xh")
            xl_t = p_x.tile([128, NKK, 2, T], f8, name="xl", tag="xl")
            wqk_t = p_w.tile([128, NKK, 2, 2 * CH], f8, name="wqk", tag="wqk")
            wv_t = p_w.tile([128, NKK, 2, 3 * CH], f8, name="wv", tag="wv")
            wp_t = p_w.tile([128, 2, C], f16, name="wp", tag="wp")
            msk_t = p_w.tile([128, 3, 128], bf16, name="msk", tag="msk")
            misc_t = p_w.tile([128, 264], f32, name="misc", tag="misc")
            bvb = p_w.tile([128, CH], f32, name="bvb", tag="bvb")
            xh = [xh_t[:, k] for k in range(NKK)]
            xl = [xl_t[:, k] for k in range(NKK)]
            wqk = [wqk_t[:, k] for k in range(NKK)]
            wvh = [wv_t[:, k, :, 0:CH] for k in range(NKK)]
            wvl = [wv_t[:, k, :, CH:2 * CH] for k in range(NKK)]
            wvh16 = [wv_t[:, k, :, 2 * CH:3 * CH] for k in range(NKK)]
            wp = [wp_t[:, k] for k in range(2)]
            tri = msk_t[:, 0]
            neg = msk_t[:, 1]
            ident = msk_t[:, 2]
            bqk = misc_t[:, 0:4]
            bvrow = misc_t[0:1, 4:4 + CH]

            # ---- persistent intermediates ------------------------------
            # q/k fp8 pair tiles: [128, 2(pair), 2(i), T]; heads (2p, 2p+1)
            # packed on partitions 0:64 / 64:128; i=1 zeroed (DoubleRow pad).
            q_all = p_qk.tile([128, 2, 2, T], f8, name="q_all", tag="q_all")
            k_all = p_qk.tile([128, 2, 2, T], f8, name="k_all", tag="k_all")
            qt = [q_all[:, p] for p in range(2)]
            kt = [k_all[:, p] for p in range(2)]
            # v tiles: [128t, 4 heads * 65] fp16 (65th col of each head = 1)
            v = [p_v.tile([128, 4 * 65], f16, name=f"v{m}", tag=f"v{m}")
                 for m in range(NT)]
            # normalized y^T fp16: tile kk holds heads (2kk, 2kk+1)
            yT = [p_y.tile([128, T], f16, name=f"yT{p}", tag=f"yT{p}")
                  for p in range(2)]

            # ---- input DMA ramp (few large DMAs: HWDGE issue is ~625ns
            # each on a shared device, so batch aggressively) -------------
            def kip(ap):
                return ap.rearrange("(k i p) c -> p k i c", k=NKK, i=2)

            nc.sync.dma_start(out=xh_t[:, :, :, 0:1024], in_=kip(xh_d[:, 0:1024]))
            nc.sync.dma_start(out=wqk_t, in_=kip(wqk_d))
            nc.sync.dma_start(out=msk_t,
                              in_=msk_d.rearrange("p (a c) -> p a c", a=3))
            nc.sync.dma_start(out=misc_t, in_=misc_d)
            # zero the i=1 halves of q/k pair tiles (DoubleRow zero padding)
            for t_ in (q_all, k_all):
                nc.sync.dma_start(
                    out=t_[:, :, 1, :],
                    in_=zero_d.rearrange("p (a c) -> p a c", a=2))
            nc.sync.dma_start(out=wv_t, in_=kip(wv3_d))
            nc.sync.dma_start(out=xl_t[:, :, :, 0:1024], in_=kip(xl_d[:, 0:1024]))
            nc.sync.dma_start(out=xh_t[:, :, :, 1024:2048],
                              in_=kip(xh_d[:, 1024:2048]))
            nc.sync.dma_start(out=xl_t[:, :, :, 1024:2048],
                              in_=kip(xl_d[:, 1024:2048]))
            nc.sync.dma_start(out=wp_t,
                              in_=wp_d.rearrange("(k p) c -> p k c", k=2))
            nc.gpsimd.partition_broadcast(bvb, bvrow)

            # ---- building blocks --------------------------------------
            def qk_chunk(mi, nj):
                """q or k channels [128mi, 128mi+128), t [512nj, +512).
                mi 0/1 -> q pair tiles, 2/3 -> k pair tiles."""
                ps = ps_mm.tile([128, 512], f32, name="ps_qk", tag="mm")
                for k in range(NKK):
                    nc.tensor.matmul(
                        ps[:, 0:512],
                        lhsT=wqk[k][:, :, 128 * mi:128 * (mi + 1)],
                        rhs=xh[k][:, :, 512 * nj:512 * (nj + 1)],
                        start=(k == 0), stop=(k == NKK - 1), perf_mode=DR)
                dst = (qt[mi] if mi < 2 else kt[mi - 2])
                nc.vector.tensor_scalar_add(
                    dst[:, 0, 512 * nj:512 * (nj + 1)], ps[:, 0:512],
                    bqk[:, mi:mi + 1])

            def v_chunk(m):
                """v rows [128m, +128), all 256 channels, 3-pass split fp8."""
                ps = ps_mm.tile([128, 512], f32, name="ps_v", tag="mm")
                for k in range(NKK):
                    nc.tensor.matmul(
                        ps[:, 0:CH],
                        lhsT=xh[k][:, :, 128 * m:128 * (m + 1)],
                        rhs=wvh[k], start=(k == 0), stop=False, perf_mode=DR)
                for k in range(NKK):
                    nc.tensor.matmul(
                        ps[:, 0:CH],
                        lhsT=xh[k][:, :, 128 * m:128 * (m + 1)],
                        rhs=wvl[k], start=False, stop=False, perf_mode=DR)
                for k in range(NKK):
                    nc.tensor.matmul(
                        ps[:, 0:CH],
                        lhsT=xl[k][:, :, 128 * m:128 * (m + 1)],
                        rhs=wvh16[k], start=False, stop=(k == NKK - 1),
                        perf_mode=DR)
                for h in range(4):
                    nc.vector.memset(v[m][:, 65 * h + 64:65 * h + 65], 1.0)
                vi = v[m].rearrange("p (h c) -> p h c", h=4)[:, :, 0:64]
                nc.vector.tensor_tensor(
                    vi, ps[:, 0:CH].rearrange("p (h c) -> p h c", h=4),
                    bvb.rearrange("p (h c) -> p h c", h=4), mybir.AluOpType.add)

            yps = {}     # (j, h) -> y psum tile

            def s_exp(j, h, mp):
                """S^T + exp for head h, q-chunk j, keytile pair (2mp, 2mp+1).
                Returns (P tile, W0 of the pair window)."""
                p2, hi = divmod(h, 2)       # pair tile index, half
                qs0, qs1 = 64 * hi, 64 * (hi + 1)
                sps = ps_s.tile([128, 2, 512], f32, name="sps", tag="s")
                rr0 = 2 * mp - 4 * j        # diag offset of first tile (<0 if off-diag)
                W = [0, 0]                  # exact causal window starts
                for u in range(2):
                    i = 2 * mp + u
                    rr = i - 4 * j
                    W[u] = max(0, 128 * rr)
                    junk = rr >= 0 and W[u] > W[0]
                    if junk:
                        # the pair-exp window starts at W[0]; initialize the
                        # causally-dead prefix [W[0], W[u]) of this bank with
                        # -1e30 (opens the bank's psum group)
                        nc.tensor.matmul(
                            sps[:, u, W[0]:W[u]],
                            lhsT=ident[:, 0:128], rhs=neg[:, 0:W[u] - W[0]],
                            start=True, stop=False)
                    nc.tensor.matmul(
                        sps[:, u, W[u]:512],
                        lhsT=kt[p2][qs0:qs1, :, 128 * i:128 * (i + 1)],
                        rhs=qt[p2][qs0:qs1, :, 512 * j + W[u]:512 * (j + 1)],
                        start=not junk, stop=(rr < 0), perf_mode=DR)
                    if rr >= 0:
                        # triangular mask on the causal boundary block
                        nc.tensor.matmul(
                            sps[:, u, W[u]:W[u] + 128],
                            lhsT=ident, rhs=tri,
                            start=False, stop=True)
                W0 = W[0]
                pt = p_p.tile([128, 2, 512], f16, name="pt", tag="pt")
                nc.scalar.activation(
                    pt[:, :, W0:512], sps[:, :, W0:512],
                    mybir.ActivationFunctionType.Exp, scale=EXP_SCALE)
                return pt, W

            def pv(j, h, mp, pt, W, last):
                """accumulate y psum for (j, h) from P pair; normalize if last."""
                if mp == 0:
                    yps[(j, h)] = ps_y.tile([65, 512], f32, name="yp", tag="y")
                yp = yps[(j, h)]
                for u in range(2):
                    i = 2 * mp + u
                    nc.tensor.matmul(
                        yp[:, W[u]:512],
                        lhsT=v[i][:, 65 * h:65 * h + 65],
                        rhs=pt[:, u, W[u]:512],
                        start=(i == 0), stop=(last and u == 1))
                if last:
                    rc = p_sm.tile([1, 512], f32, name="rc", tag=f"rc{h % 2}")
                    bc = p_sm.tile([64, 512], f32, name="bc", tag=f"bc{h % 2}")
                    nc.vector.reciprocal(rc, yp[64:65, :])
                    nc.gpsimd.partition_broadcast(bc, rc[0:1, :])
                    p2, hi = divmod(h, 2)
                    nc.vector.tensor_tensor(
                        yT[p2][64 * hi:64 * (hi + 1), 512 * j:512 * (j + 1)],
                        yp[0:64, :], bc, mybir.AluOpType.mult)

            def proj(m):
                """output rows [128m, +128): 2 matmuls per 512-col half."""
                st = p_st.tile([128, 1024], f16, name="st", tag="st")
                for u in range(2):
                    ps = ps_mm.tile([128, 512], f32, name="ps_pr", tag="mm")
                    for kk in range(2):
                        nc.tensor.matmul(
                            ps[:, 0:512],
                            lhsT=yT[kk][:, 128 * m:128 * (m + 1)],
                            rhs=wp[kk][:, 512 * u:512 * (u + 1)],
                            start=(kk == 0), stop=(kk == 1))
                    if u == 0:
                        nc.vector.tensor_copy(st[:, 0:512], ps[:, 0:512])
                    else:
                        nc.scalar.copy(st[:, 512:1024], ps[:, 0:512])
                nc.gpsimd.dma_start(out=out_d[128 * m:128 * (m + 1), :], in_=st)

            # ---- emission schedule ------------------------------------
            fillers = []

            def fill():
                for _ in range(2):
                    if fillers:
                        fillers.pop(0)()

            # wave 0 q/k chunks ahead of attention(0); v tiles 0..3 go into
            # the j=0 filler queue (PV trails exp by one pair anyway)
            for mi in (0, 2, 1, 3):
                qk_chunk(mi, 0)

            PAIRS = [(j, h, mp)
                     for j in range(NJ) for h in range(4) for mp in range(2 * j + 2)]
            pending = None
            cur_j = -1
            for (j, h, mp) in PAIRS:
                if j != cur_j:
                    while fillers:
                        fillers.pop(0)()
                    cur_j = j
                    if j == 0:       # v tiles for j=0 (PV trails by a pair)
                        for m in range(4):
                            fillers.append(lambda m=m: v_chunk(m))
                    if j + 1 < NJ:   # next wave's qkv as fillers
                        for mi in (0, 2, 1, 3):
                            fillers.append(
                                lambda mi=mi, nj=j + 1: qk_chunk(mi, nj))
                        for m in range(4 * (j + 1), 4 * (j + 2)):
                            fillers.append(lambda m=m: v_chunk(m))
                    if j >= 1:       # proj of the previous chunk
                        for m in range(4 * (j - 1), 4 * j):
                            fillers.append(lambda m=m: proj(m))
                pt_w = s_exp(j, h, mp)
                if pending is not None:
                    pj, ph, pmp, ppt, pW = pending
                    pv(pj, ph, pmp, ppt, pW, last=(pmp == 2 * pj + 1))
                fill()
                pending = (j, h, mp, pt_w[0], pt_w[1])
            pj, ph, pmp, ppt, pW = pending
            pv(pj, ph, pmp, ppt, pW, last=True)
            while fillers:
                fillers.pop(0)()
            for m in range(12, 16):
                proj(m)

    nc.compile()
    return nc


def _host_inputs(x, W_attn, b_attn, W_proj):
    """Build the 8 per-core input maps (numpy only)."""
    import ml_dtypes
    e4 = ml_dtypes.float8_e4m3
    bf = ml_dtypes.bfloat16

    x = np.asarray(x, dtype=np.float32)
    W_attn = np.asarray(W_attn, dtype=np.float32)
    b_attn = np.asarray(b_attn, dtype=np.float32)
    W_proj = np.asarray(W_proj, dtype=np.float32)

    # causal triangular block: [k_local p, q_local c] = 0 if c >= p else -1e30
    cc = np.arange(128)[None, :]
    pp = np.arange(128)[:, None]
    tri = np.where(cc >= pp, 0.0, -1e30).astype(bf)
    neg = np.full((128, 128), -1e30, np.float32).astype(bf)
    ident = np.eye(128, dtype=bf)
    msk = np.concatenate([tri, neg, ident], axis=1)
    zero = np.zeros((128, 2 * T), dtype=e4)

    in_maps = []
    for c in range(NCORES):
        b, g = divmod(c, 4)
        sl = slice(CH * g, CH * (g + 1))
        xb = np.ascontiguousarray(x[b].T)                    # [C, T]
        xh = xb.astype(e4)
        xl = ((xb - xh.astype(np.float32)) * 16.0).astype(e4)
        wq = 32.0 * W_attn[:, 0 * C:1 * C][:, sl]
        wk = 32.0 * W_attn[:, 1 * C:2 * C][:, sl]
        wv = 32.0 * W_attn[:, 2 * C:3 * C][:, sl]
        wqk = np.concatenate([wq, wk], axis=1).astype(e4)
        wvh = wv.astype(e4)
        wvl = (wv - wvh.astype(np.float32)).astype(e4)
        wvh16 = (wvh.astype(np.float32) / 16.0).astype(e4)
        wv3 = np.concatenate([wvh, wvl, wvh16], axis=1)
        wp = (32.0 * W_proj[sl, :]).astype(np.float16)
        bq = 32.0 * b_attn[0 * C:1 * C][sl]
        bk = 32.0 * b_attn[1 * C:2 * C][sl]
        bv = 32.0 * b_attn[2 * C:3 * C][sl]
        bqk = np.stack([bq[0:128], bq[128:256], bk[0:128], bk[128:256]], axis=1)
        misc = np.zeros((128, 264), np.float32)
        misc[:, 0:4] = bqk
        misc[0, 4:4 + CH] = bv
        in_maps.append({
            "xh": xh, "xl": xl,
            "wqk": np.ascontiguousarray(wqk),
            "wv3": np.ascontiguousarray(wv3),
            "wp": np.ascontiguousarray(wp),
            "misc": misc,
            "msk": msk, "zero": zero,
        })
    return in_maps


def kernel(x, W_attn, b_attn, W_proj, b_proj, _want_results=None):
    global _COMPILED
    from concourse.bass_utils import run_bass_kernel_spmd

    if _COMPILED is None:
        _COMPILED = _build()
    nc = _COMPILED

    in_maps = _host_inputs(x, W_attn, b_attn, W_proj)
    kw = dict(_want_results or {})
    res = run_bass_kernel_spmd(nc, in_maps, core_ids=list(range(NCORES)), **kw)
    if _want_results is not None:
        kernel.last_results = res

    out = np.zeros((B, T, C), dtype=np.float32)
    for c in range(NCORES):
        out[c // 4] += np.asarray(res.results[c]["out_p"], dtype=np.float32)
    out *= 1.0 / 1024.0
    out += np.asarray(b_proj, dtype=np.float32)[None, None, :]
    return out
